# revision 28
# baseline (speedup 1.0000x reference)
"""Trainium2 Bass kernel for nn_MultiHeadAttention_88854283419963 (TriAffine attention).

8 NeuronCores, SPMD.  The TriAffine contraction
    s[b,x,y,z,r] = sum_{i,k,j} xaug[b,x,i] mid[b,z,k] Wtri[i,k,j,r] yaug[b,y,j]
is factored k -> i -> j.  Wtri is sharded along j (48 j's per core).

v8 (v6 538us -> v7 458us -> this):
  - W streamed with ONE contiguous DMA per chunk (host pre-tiled), 5-chunk
    prefetch so MM1 rides through HBM contention from concurrent AllToAlls.
  - u staged to the AllToAll inputs incrementally from a small rotating
    buffer (dest-major x-minor == partition order -> linear [128,..] DMA).
    FOUR pipelined AllToAlls triggered after chunks 6/12/18/24, so the
    collective stream starts at ~35% of stage C and the last (quarter-
    size) piece is the only exposed tail.
  - All MLPs (incl. tail) + the A_r chain run in stage A with weights
    loaded upfront -- v7 lost ~50us loading tail-MLP weights through
    a2a-saturated HBM at stage-C end.
  - Stage E/F: the t_bias/corner correction T[y,z] is folded into MM3 as
    3 extra accumulating matmuls (lhsT=A_r, rhs=mid broadcast over x), so
    softmax reads PSUM directly; Vw is folded into a per-r sign-sorted,
    |Vw|-scaled copy of text_embeddings so the post-MM4 reduction is two
    relu-accumulates straight from PSUM (relu(S*|w|)=|w|*relu(S)), the
    [128,768] relu+dot chain of v6/v7 is gone.  The transpose scratch
    lives in ps4's spare columns (no extra PSUM bank).
  - q/k/p_attn fills the a2a-tail bubble between the two E/F halves.
  - Stage G runs all 16 (b,h) panes in [128,2048]-wide batched ops.
"""

import sys

sys.path.insert(0, "/opt/trn_rl_repo")
sys.path.insert(0, "/root/.axon_site/_ro/trn_rl_repo")

import math

import numpy as np

import concourse.bass as bass
import concourse.mybir as mybir
from concourse.masks import make_identity
from concourse.tile import TileContext
from bass_rust import ScopedClock

# ----------------------------------------------------------------------------
# Workaround: this container's walrus build rejects >1 sync-wait on the CTRL
# (Drain) instruction Tile emits at the kernel tail ("Too many sync wait
# commands").  Split the waits across single-wait NOPs instead.
# ----------------------------------------------------------------------------


def _patched_drain_and_barrier(self, tick_clock, wait_clock):
    probe = self.nc.sync.nop()
    wait_clock.add_sem_waits(probe.ins, ScopedClock({None: tick_clock.global_clock}))
    si = probe.ins.sync_info
    if si is not None and len(si.on_wait) > 1:
        waits = list(si.on_wait)
        probe.ins.sync_info = mybir.SyncInfo(
            on_wait=[waits[0]], on_update=list(si.on_update)
        )
        for w in waits[1:]:
            extra = self.nc.sync.nop()
            extra.ins.sync_info = mybir.SyncInfo(on_wait=[w], on_update=[])
    self.nc.sync.drain()
    self.nc.all_engine_barrier()
    assert self.sems is not None
    popped = self.nc._tile_sem_poison_stack.pop()
    assert popped is self._sem_poison
    self.nc.clear_and_free_semaphores(list(self.sems.allocated().values()))
    self.nc.all_engine_barrier()


TileContext._drain_and_barrier = _patched_drain_and_barrier

_NOPN = [0]


def _split_multiwaits(nc, limit=1):
    """walrus in this container accepts at most one sync-wait per instruction;
    move extra waits onto same-engine NoOps inserted just before."""
    for f in nc.m.functions:
        for blk in f.blocks:
            changed = False
            new = []
            for inst in blk.instructions:
                si = getattr(inst, "sync_info", None)
                if si is not None and len(si.on_wait) > limit:
                    ow = list(si.on_wait)
                    for w in ow[:-limit]:
                        _NOPN[0] += 1
                        nop = mybir.InstNoOp(name=f"mwsplit_{_NOPN[0]}", ins=[], outs=[])
                        nop.engine = inst.engine
                        nop.sync_info = mybir.SyncInfo(on_wait=[w], on_update=[])
                        new.append(nop)
                    inst.sync_info = mybir.SyncInfo(
                        on_wait=ow[-limit:], on_update=list(si.on_update)
                    )
                    changed = True
                new.append(inst)
            if changed:
                blk.instructions = new

# ----------------------------------------------------------------------------
B, L, D = 2, 128, 768
H, DK = 4, 192
HD, NC = 384, 2
CORES = 8
JC = HD // CORES          # 48
XL = L // CORES           # 16
NJR = 2 * JC              # 96
BL = B * L                # 256

F32 = mybir.dt.float32
F16 = mybir.dt.float16

DT_STORE = F16
W_CHUNK = 4               # jr's per streamed W chunk
NCH = NJR // W_CHUNK      # 24 full chunks after the uex chunk

# AllToAll pieces (trigger after these chunks):
# chunk 0 = uex (2 rows); chunks 1..12 = r0 jj 0..47; 13..24 = r1 jj 0..47
PIECES = [
    (4, 2 + 12),   # uex + r0 jj 0..11  (small: starts the wire early)
    (12, 36),      # r0 jj 12..47
    (18, 24),      # r1 jj 0..23
    (24, 24),      # r1 jj 24..47
]

A = mybir.ActivationFunctionType
Alu = mybir.AluOpType
Ax = mybir.AxisListType


def build(npos, debug=False):
    """npos: (npos_r0, npos_r1) -- column counts of the positive-|Vw| halves
    of the per-r sign-sorted text embeddings."""
    nc = bass.Bass(num_devices=CORES)

    # ---- inputs ----
    teT = nc.dram_tensor("teT", [128, 6, BL], DT_STORE, kind="ExternalInput")
    # per-r sign-sorted, |Vw|-scaled text embeddings: [z, b, r, h']
    terd = nc.dram_tensor("ter", [128, B, NC, D], DT_STORE, kind="ExternalInput")
    mlp_in = {}
    for nm in ("h", "m", "t"):
        mlp_in[nm] = (
            nc.dram_tensor(f"W{nm}1", [128, 6, HD], DT_STORE, kind="ExternalInput"),
            nc.dram_tensor(f"b{nm}1", [128, 3], F32, kind="ExternalInput"),
            nc.dram_tensor(f"W{nm}2", [128, 3, HD], DT_STORE, kind="ExternalInput"),
            nc.dram_tensor(f"b{nm}2", [128, 3], F32, kind="ExternalInput"),
        )

    Wq = nc.dram_tensor("Wq", [128, 6, D], DT_STORE, kind="ExternalInput")
    bq = nc.dram_tensor("bq", [128, 6], F32, kind="ExternalInput")
    Wk = nc.dram_tensor("Wk", [128, 6, D], DT_STORE, kind="ExternalInput")
    bk = nc.dram_tensor("bk", [128, 6], F32, kind="ExternalInput")
    qryT = nc.dram_tensor("qryT", [128, 6, BL], DT_STORE, kind="ExternalInput")
    keyT = nc.dram_tensor("keyT", [128, 6, BL], DT_STORE, kind="ExternalInput")
    pmask = nc.dram_tensor("pmask", [128, B, L], F32, kind="ExternalInput")

    # W chunks, host-tiled: one contiguous [128, 3*g*HD] DMA each
    W1c0 = nc.dram_tensor("W1c0", [128, 3, 2, HD], DT_STORE, kind="ExternalInput")
    W1c = nc.dram_tensor("W1c", [NCH, 128, 3, W_CHUNK, HD], DT_STORE,
                         kind="ExternalInput")
    WbiT = nc.dram_tensor("WbiT", [128, 3, 2, HD], DT_STORE, kind="ExternalInput")
    Wlast = nc.dram_tensor("Wlast", [1, 2, HD], DT_STORE, kind="ExternalInput")
    ones_in = nc.dram_tensor("ones16", [1, BL], DT_STORE, kind="ExternalInput")

    madd = nc.dram_tensor("madd", [128, XL, 128], F32, kind="ExternalInput")
    Vb_in = nc.dram_tensor("Vb", [2, 1], F32, kind="ExternalInput")

    out = nc.dram_tensor("out", [B, H, L, L], F32, kind="ExternalOutput")

    # a2a pieces.  Inputs are declared [(dest x)=128, b, rows, z]:
    # dest-major x-minor matches the partition order of the u tiles, so
    # staging is a linear [128,...] DMA and the collective's linear
    # dest-chunking still sees [dest][16x, b, rows, z].
    a2a_in, a2a_out = [], []
    for pi, (_, rows) in enumerate(PIECES):
        a2a_in.append(
            nc.dram_tensor(f"a2a{pi}_in", [128, B, rows, 128], DT_STORE)
        )
        a2a_out.append(
            nc.dram_tensor(f"a2a{pi}_out", [CORES, XL, B, rows, 128], DT_STORE)
        )
    # score AG payload per r: [xl, y2, b, y1]  (y = y2*32 + y1)
    ag_in = [nc.dram_tensor(f"ag_in{r}", [XL, 4, B, 32], F32) for r in range(NC)]
    ag_out = [
        nc.dram_tensor(f"ag_out{r}", [CORES, XL, 4, B, 32], F32,
                       addr_space="Shared")
        for r in range(NC)
    ]

    # (tile_idx, p0, src_core, jj0, n): partition runs for R3 gathers with a
    # restricted jj range [lo, hi) per source core
    def r3_slices(lo, hi):
        runs = []
        for s in range(CORES):
            jj0 = lo
            while jj0 < hi:
                gl = s * JC + jj0
                t, p0 = gl // 128, gl % 128
                n = min(128 - p0, hi - jj0)
                runs.append((t, p0, s, jj0, n))
                jj0 += n
        return runs

    with TileContext(nc) as tc:
        with (
            tc.tile_pool(name="res", bufs=1) as res,
            tc.tile_pool(name="res16", bufs=1) as res16,
        ):
            ident32 = res.tile([128, 128], F32)
            make_identity(nc, ident32)

            headT16 = res16.tile([128, 3, BL], DT_STORE, name="headT16")
            midT16 = res16.tile([128, 3, BL], DT_STORE, name="midT16")
            tailT16 = res16.tile([128, 3, BL], DT_STORE, name="tailT16")
            pattn = res.tile([128, B * H, L], F32, name="pattn")
            score_r = [
                res.tile([128, B * XL], F32, name=f"score_r{r}") for r in range(NC)
            ]
            A_sb = res16.tile([128, 3, 2, BL], DT_STORE, name="A_sb")
            ones_sb = res16.tile([1, BL], DT_STORE, name="ones16")
            ter_sb = res16.tile([128, B, NC, D], DT_STORE, name="ter")
            vbb = res.tile([128, 2], F32)
            madd_sb = res.tile([128, XL, 128], F32)
            ones_col = res.tile([128, 1], F32, name="ones_col")

            # R3 (j-on-partition u tiles) + uex rows live into stage E/F
            R3_r0, R3_r1, uexrows = {}, {}, {}
            for b in range(B):
                R3_r0[b] = res16.tile([128, 3, XL * 128], DT_STORE, name=f"R3a{b}")
                R3_r1[b] = res16.tile([128, 3, XL * 128], DT_STORE, name=f"R3b{b}")
                for r in range(NC):
                    uexrows[(r, b)] = res16.tile(
                        [1, XL * 128], DT_STORE, name=f"uexr_{r}{b}"
                    )

            # ============ stage A: mid/head MLPs (tail deferred to C end) ====
            stackA = __import__("contextlib").ExitStack()
            mlpw0 = stackA.enter_context(tc.tile_pool(name="mlpw0", bufs=1))
            psA = stackA.enter_context(tc.tile_pool(name="psA", bufs=2, space="PSUM"))
            tmpA = stackA.enter_context(tc.tile_pool(name="tmpA", bufs=2))
            if True:
                teT_sb = mlpw0.tile([128, 6, BL], DT_STORE, name="teT_sb")
                nc.sync.dma_start(teT_sb[:], teT[:])
                mlpw = {}
                for nm in ("m", "h", "t"):
                    W1d, b1d, W2d, b2d = mlp_in[nm]
                    eng = nc.scalar if nm == "t" else nc.sync
                    w1 = mlpw0.tile([128, 6, HD], DT_STORE, name=f"w1_{nm}")
                    eng.dma_start(w1[:], W1d[:])
                    w2 = mlpw0.tile([128, 3, HD], DT_STORE, name=f"w2_{nm}")
                    eng.dma_start(w2[:], W2d[:])
                    b1 = mlpw0.tile([128, 3], F32, name=f"b1_{nm}")
                    eng.dma_start(b1[:], b1d[:])
                    b2 = mlpw0.tile([128, 3], F32, name=f"b2_{nm}")
                    eng.dma_start(b2[:], b2d[:])
                    mlpw[nm] = (w1, b1, w2, b2)
                wbiT_sb = mlpw0.tile([128, 3, 2, HD], DT_STORE, name="wbiT")
                nc.scalar.dma_start(wbiT_sb[:], WbiT[:])
                wlast_sb = mlpw0.tile([1, 2, HD], DT_STORE, name="wlast")
                nc.scalar.dma_start(wlast_sb[:], Wlast[:])
                # small E/F-time loads (after the compute-critical ones)
                nc.scalar.dma_start(ter_sb[:], terd[:])
                for r in range(2):
                    nc.scalar.dma_start(
                        vbb[:, r : r + 1], Vb_in[r : r + 1, :].to_broadcast([128, 1])
                    )
                nc.scalar.dma_start(madd_sb[:], madd[:])
                nc.scalar.dma_start(ones_sb[:], ones_in[:])
                nc.vector.memset(ones_col[:], 1.0)

                for nm, dst in (("m", midT16), ("h", headT16)):
                    w1, b1, w2, b2 = mlpw[nm]
                    h1 = tmpA.tile([128, 3, BL], DT_STORE, name="h1")
                    for mt in range(3):
                        ps = psA.tile([128, BL], F32, name="psA")
                        for ks in range(6):
                            nc.tensor.matmul(
                                ps[:], w1[:, ks, mt * 128 : (mt + 1) * 128],
                                teT_sb[:, ks, :], start=(ks == 0), stop=(ks == 5),
                            )
                        nc.scalar.activation(
                            h1[:, mt, :], ps[:], A.Relu, bias=b1[:, mt : mt + 1]
                        )
                    for mt in range(3):
                        ps = psA.tile([128, BL], F32, name="psA")
                        for ks in range(3):
                            nc.tensor.matmul(
                                ps[:], w2[:, ks, mt * 128 : (mt + 1) * 128],
                                h1[:, ks, :], start=(ks == 0), stop=(ks == 2),
                            )
                        nc.scalar.activation(
                            dst[:, mt, :], ps[:], A.Identity, bias=b2[:, mt : mt + 1]
                        )


            # ================= stage C: chunk loop (MM1 + MM2) =================
            # piece-relative row offset for chunk ch (ch0 = 2 uex rows,
            # ch k>=1 = rows 2+(k-1)*4 .. +4 in global [uex|r0|r1] order)
            piece_start = [0]
            for _, rows in PIECES:
                piece_start.append(piece_start[-1] + rows)

            with (
                tc.tile_pool(name="urot", bufs=8) as urot,
                tc.tile_pool(name="wchunk", bufs=5) as wchunk,
                tc.tile_pool(name="tbig", bufs=2) as tbigp,
                tc.tile_pool(name="psU", bufs=2, space="PSUM") as psU,
                tc.tile_pool(name="psT", bufs=2, space="PSUM") as psT,
            ):
                for ch in range(NCH + 1):
                    wt = wchunk.tile([128, 3, W_CHUNK, HD], DT_STORE, name="wt")
                    if ch == 0:
                        g = 2
                        nc.sync.dma_start(wt[:, :, :2, :], W1c0[:])
                    else:
                        g = W_CHUNK
                        nc.sync.dma_start(wt[:], W1c[ch - 1])

                    # MM1: t_big[i, it, jl, (b z)]
                    t_big = tbigp.tile([128, 3, W_CHUNK, BL], DT_STORE, name="t_big")
                    for jl in range(g):
                        ps = psT.tile([128, 3, BL], F32, name="psT")
                        for it in range(3):
                            for ks in range(3):
                                nc.tensor.matmul(
                                    ps[:, it, :],
                                    wt[:, ks, jl, it * 128 : (it + 1) * 128],
                                    midT16[:, ks, :],
                                    start=(ks == 0), stop=(ks == 2),
                                )
                        nc.vector.tensor_copy(t_big[:, :, jl, :], ps[:])

                    # MM2: u[x, (jl z)] per b into a small rotating buffer
                    # (its only reader is the staging DMA right below)
                    uch = urot.tile([128, B, W_CHUNK, 128], DT_STORE, name="uch")
                    for b in range(B):
                        psu = psU.tile([128, W_CHUNK * 128], F32, name="psU")
                        rhs_n = g * 128
                        for it in range(3):
                            nc.tensor.matmul(
                                psu[:, :rhs_n],
                                headT16[:, it, b * L : (b + 1) * L],
                                t_big[:, it, :g, b * L : (b + 1) * L],
                                start=(it == 0), stop=(it == 2),
                            )
                        nc.vector.tensor_copy(
                            uch[:, b, :g, :],
                            psu[:, : g * 128].rearrange("p (g z) -> p g z", z=128),
                        )

                    # ---- incremental a2a staging (pool queue) ----
                    grow = 0 if ch == 0 else 2 + (ch - 1) * W_CHUNK
                    pi = max(i for i in range(len(PIECES)) if piece_start[i] <= grow)
                    rr = grow - piece_start[pi]
                    nc.gpsimd.dma_start(
                        a2a_in[pi][:, :, rr : rr + g, :],
                        uch[:, :, :g, :],
                    )

                    # ---- piece triggers + receive-side gathers ----
                    for pi2, (tch, _) in enumerate(PIECES[:-1]):
                        if ch == tch:
                            nc.gpsimd.collective_compute(
                                "AllToAll", Alu.bypass,
                                replica_groups=[list(range(CORES))],
                                ins=[a2a_in[pi2][:]], outs=[a2a_out[pi2][:]],
                            )
                            # r0 gathers on scalar (idle during stage C)
                            if pi2 == 0:
                                for b in range(B):
                                    for r in range(NC):
                                        nc.scalar.dma_start(
                                            uexrows[(r, b)][:].rearrange(
                                                "c (x z) -> c x z", z=128
                                            ),
                                            a2a_out[0][0:1, :, b, r, :],
                                        )
                                for b in range(B):
                                    for t, p0, s, jj0, n in r3_slices(0, 12):
                                        nc.scalar.dma_start(
                                            R3_r0[b][p0 : p0 + n, t, :].rearrange(
                                                "p (x z) -> p x z", z=128
                                            ),
                                            a2a_out[0][
                                                s, :, b, 2 + jj0 : 2 + jj0 + n, :
                                            ].rearrange("x j z -> j x z"),
                                        )
                            if pi2 == 1:
                                for b in range(B):
                                    for t, p0, s, jj0, n in r3_slices(12, JC):
                                        nc.sync.dma_start(
                                            R3_r0[b][p0 : p0 + n, t, :].rearrange(
                                                "p (x z) -> p x z", z=128
                                            ),
                                            a2a_out[1][
                                                s, :, b, jj0 - 12 : jj0 - 12 + n, :
                                            ].rearrange("x j z -> j x z"),
                                        )

                # last piece: trigger + r1 gathers (pool; nothing queued after)
                nc.gpsimd.collective_compute(
                    "AllToAll", Alu.bypass,
                    replica_groups=[list(range(CORES))],
                    ins=[a2a_in[3][:]], outs=[a2a_out[3][:]],
                )
                for b in range(B):
                    for t, p0, s, jj0, n in r3_slices(0, 24):
                        nc.sync.dma_start(
                            R3_r1[b][p0 : p0 + n, t, :].rearrange(
                                "p (x z) -> p x z", z=128
                            ),
                            a2a_out[2][
                                s, :, b, jj0 : jj0 + n, :
                            ].rearrange("x j z -> j x z"),
                        )
                for b in range(B):
                    for t, p0, s, jj0, n in r3_slices(24, JC):
                        nc.sync.dma_start(
                            R3_r1[b][p0 : p0 + n, t, :].rearrange(
                                "p (x z) -> p x z", z=128
                            ),
                            a2a_out[3][
                                s, :, b, jj0 - 24 : jj0 - 24 + n, :
                            ].rearrange("x j z -> j x z"),
                        )

            # ---- deferred tail MLP + A_r chain (PE tail of stage C) ----
            w1, b1, w2, b2 = mlpw['t']
            h1 = tmpA.tile([128, 3, BL], DT_STORE, name="h1")
            for mt in range(3):
                ps = psA.tile([128, BL], F32, name="psA")
                for ks in range(6):
                    nc.tensor.matmul(
                        ps[:], w1[:, ks, mt * 128 : (mt + 1) * 128],
                        teT_sb[:, ks, :], start=(ks == 0), stop=(ks == 5),
                    )
                nc.scalar.activation(
                    h1[:, mt, :], ps[:], A.Relu, bias=b1[:, mt : mt + 1]
                )
            for mt in range(3):
                ps = psA.tile([128, BL], F32, name="psA")
                for ks in range(3):
                    nc.tensor.matmul(
                        ps[:], w2[:, ks, mt * 128 : (mt + 1) * 128],
                        h1[:, ks, :], start=(ks == 0), stop=(ks == 2),
                    )
                nc.scalar.activation(
                    tailT16[:, mt, :], ps[:], A.Identity, bias=b2[:, mt : mt + 1]
                )
            # A_r[k,y] = sum_j WbiT[j,k,r] tail_aug[j,y]  (T-fold lhsT)
            for r in range(2):
                for kt in range(3):
                    ps = psA.tile([128, BL], F32, name="psA")
                    for jt in range(3):
                        nc.tensor.matmul(
                            ps[:], wbiT_sb[:, jt, r, kt * 128 : (kt + 1) * 128],
                            tailT16[:, jt, :], start=(jt == 0), stop=False,
                        )
                    nc.tensor.matmul(
                        ps[:], wlast_sb[:, r, kt * 128 : (kt + 1) * 128],
                        ones_sb[:], start=False, stop=True,
                    )
                    nc.scalar.activation(A_sb[:, kt, r, :], ps[:], A.Copy)
            stackA.close()

            # ============ stage E/F: MM3(+T) + softmax-z + MM4 + score ============
            def ef_half(r, postp, post16, ps4p, psSp):
                accP = postp.tile([128, B * XL], F32, name="accP")
                accN = postp.tile([128, B * XL], F32, name="accN")
                esum_all = postp.tile([128, B * XL], F32, name="esumA")
                for b in range(B):
                    R3 = R3_r0[b] if r == 0 else R3_r1[b]
                    uexrow = uexrows[(r, b)]
                    for hh in range(2):
                        h0 = hh * 1024
                        psS = psSp.tile([128, 1024], F32, name="psS")
                        for q in range(2):
                            c0 = h0 + q * 512
                            l0 = q * 512
                            for t in range(3):
                                nc.tensor.matmul(
                                    psS[:, l0 : l0 + 512],
                                    tailT16[:, t, b * L : (b + 1) * L],
                                    R3[:, t, c0 : c0 + 512],
                                    start=(t == 0), stop=False,
                                )
                            # T-fold: psS[y, xz] += sum_k A_r[k,y] mid[z,k]
                            # (mid broadcast over the 4 x's of this segment)
                            for kt in range(3):
                                nc.tensor.matmul(
                                    psS[:, l0 : l0 + 512].rearrange(
                                        "p (x z) -> p x z", z=128
                                    ),
                                    A_sb[:, kt, r, b * L : (b + 1) * L],
                                    midT16[:, kt, b * L : (b + 1) * L][
                                        :, None, :
                                    ].broadcast_to([128, 4, 128]),
                                    start=False, stop=False,
                                )
                            nc.tensor.matmul(
                                psS[:, l0 : l0 + 512], ones_sb[:, 0:128],
                                uexrow[:, c0 : c0 + 512],
                                start=False, stop=True,
                            )
                        for xl0 in range(hh * 8, hh * 8 + 8, 2):
                          c0 = (xl0 - hh * 8) * 128
                          # clip-mask reads PSUM directly; min(s, +-1e6) gives
                          # EXACTLY -1e6 on masked entries
                          sm2 = postp.tile([128, 2, 128], F32, name="sm2")
                          nc.vector.tensor_tensor(
                              sm2[:],
                              psS[:, c0 : c0 + 256].rearrange(
                                  "p (u z) -> p u z", z=128
                              ),
                              madd_sb[:, xl0 : xl0 + 2, :], Alu.min,
                          )
                          # per-xl max bias: fully-masked rows (y < x,
                          # all entries -1e6) must softmax to exactly
                          # uniform, so the bias cannot come from the
                          # sibling xl
                          mx2 = postp.tile([128, 2], F32, name="mx2")
                          nc.vector.tensor_reduce(
                              mx2[:], sm2[:], Ax.X, Alu.max, negate=True
                          )
                          e2 = postp.tile([128, 2, 128], F32, name="e2")
                          for u_i in range(2):
                              nc.scalar.activation(
                                  e2[:, u_i, :], sm2[:, u_i, :],
                                  A.Exp, bias=mx2[:, u_i : u_i + 1],
                              )
                          nc.vector.tensor_reduce(
                              esum_all[:, b * XL + xl0 : b * XL + xl0 + 2],
                              e2[:], Ax.X, Alu.add,
                          )
                          for xl in (xl0, xl0 + 1):
                            u_i = xl - xl0
                            # ps4 cols 768:896 double as transpose scratch
                            ps4 = ps4p.tile([128, 1024], F32, name="ps4")
                            nc.tensor.transpose(
                                ps4[:, 768:896], e2[:, u_i, :], ident32[:]
                            )
                            eT = post16.tile([128, 128], DT_STORE, name="eT")
                            if xl % 2 == 0:
                                nc.scalar.activation(eT[:], ps4[:, 768:896], A.Copy)
                            else:
                                nc.vector.tensor_copy(eT[:], ps4[:, 768:896])
                            nc.tensor.matmul(
                                ps4[:, 0:512], eT[:], ter_sb[:, b, r, 0:512],
                                start=True, stop=True,
                            )
                            nc.tensor.matmul(
                                ps4[:, 512:768], eT[:], ter_sb[:, b, r, 512:768],
                                start=True, stop=True,
                            )
                            # sign-trick: score = (sum_pos relu - sum_neg relu)
                            # / esum + Vb, straight from PSUM
                            npos_r = npos[r]
                            col = b * XL + xl
                            jkp = post16.tile([128, D], DT_STORE, name="jkp")
                            nc.scalar.activation(
                                jkp[:, 0:npos_r],
                                ps4[:, 0:npos_r], A.Relu,
                                accum_out=accP[:, col : col + 1],
                            )
                            jkn = post16.tile([128, D], DT_STORE, name="jkn")
                            nc.vector.scalar_tensor_tensor(
                                jkn[:, 0 : D - npos_r],
                                ps4[:, npos_r:D], 0.0,
                                ones_col[:, 0:1].broadcast_to(
                                    [128, D - npos_r]
                                ),
                                Alu.max, Alu.mult,
                                accum_out=accN[:, col : col + 1],
                            )

                # batched score finalization: 4 wide ops for all 32 cols
                dif = postp.tile([128, B * XL], F32, name="dif")
                nc.vector.tensor_tensor(dif[:], accP[:], accN[:], Alu.subtract)
                recA = postp.tile([128, B * XL], F32, name="recA")
                nc.vector.reciprocal(recA[:], esum_all[:])
                nc.vector.tensor_tensor(dif[:], dif[:], recA[:], Alu.mult)
                nc.vector.tensor_scalar(
                    score_r[r][:], dif[:], vbb[:, r : r + 1], None, Alu.add
                )

                # fire this r-half's score AllGather as soon as done.
                # payload [xl, y2, b, y1]: score cols (b xl) -> partitions
                # via PE transpose, then 2 DMAs scatter y into (y2, y1)
                psS = psSp.tile([128, 1024], F32, name="psS")
                nc.tensor.transpose(
                    psS[0 : B * XL, 0:128], score_r[r][:], ident32[:],
                )
                sc_t = postp.tile([B * XL, 128], F32, name="sc_t")
                nc.vector.tensor_copy(sc_t[:], psS[0 : B * XL, 0:128])
                for b in range(B):
                    nc.scalar.dma_start(
                        ag_in[r][:, :, b, :],
                        sc_t[b * XL : (b + 1) * XL, :].rearrange(
                            "x (y2 y1) -> x y2 y1", y1=32
                        ),
                    )
                nc.gpsimd.collective_compute(
                    "AllGather", Alu.bypass,
                    replica_groups=[list(range(CORES))],
                    ins=[ag_in[r][:]], outs=[ag_out[r][:]],
                )

            with (
                tc.tile_pool(name="postp0", bufs=4) as postp,
                tc.tile_pool(name="post160", bufs=2) as post16,
                tc.tile_pool(name="ps40", bufs=3, space="PSUM") as ps4p,
                tc.tile_pool(name="psS0", bufs=1, space="PSUM") as psSp,
            ):
                ef_half(0, postp, post16, ps4p, psSp)

            with (
                tc.tile_pool(name="postp1", bufs=4) as postp,
                tc.tile_pool(name="post161", bufs=2) as post16,
                tc.tile_pool(name="ps41", bufs=3, space="PSUM") as ps4p,
                tc.tile_pool(name="psS1", bufs=1, space="PSUM") as psSp,
            ):
                ef_half(1, postp, post16, ps4p, psSp)

            # ---- q/k + p_attn: overlaps the score AllGathers / stage-G waits ----
            with (
                tc.tile_pool(name="qkw", bufs=1) as qkw,
                tc.tile_pool(name="psQ", bufs=2, space="PSUM") as psQ,
                tc.tile_pool(name="tmpQ", bufs=2) as tmpQ,
            ):
                wq_sb = qkw.tile([128, 6, D], DT_STORE, name="wqk")
                nc.sync.dma_start(wq_sb[:], Wq[:])
                bq_sb = qkw.tile([128, 6], F32, name="bqs")
                nc.sync.dma_start(bq_sb[:], bq[:])
                wk_sb = qkw.tile([128, 6, D], DT_STORE, name="wqk")
                nc.sync.dma_start(wk_sb[:], Wk[:])
                bk_sb = qkw.tile([128, 6], F32, name="bks")
                nc.sync.dma_start(bk_sb[:], bk[:])
                qT_sb = qkw.tile([128, 6, BL], DT_STORE, name="qkT")
                nc.sync.dma_start(qT_sb[:], qryT[:])
                kT_sb = qkw.tile([128, 6, BL], DT_STORE, name="qkT")
                nc.sync.dma_start(kT_sb[:], keyT[:])
                pm_sb = qkw.tile([128, B, L], F32, name="pm")
                nc.sync.dma_start(pm_sb[:], pmask[:])

                qpT = qkw.tile([128, 6, BL], DT_STORE, name="qpT")
                kpT = qkw.tile([128, 6, BL], DT_STORE, name="kpT")
                for wmat, bvec, src, dst2 in (
                    (wq_sb, bq_sb, qT_sb, qpT),
                    (wk_sb, bk_sb, kT_sb, kpT),
                ):
                    for mt in range(6):
                        ps = psQ.tile([128, BL], F32, name="psQ")
                        for ks in range(6):
                            nc.tensor.matmul(
                                ps[:],
                                wmat[:, ks, mt * 128 : (mt + 1) * 128],
                                src[:, ks, :],
                                start=(ks == 0), stop=(ks == 5),
                            )
                        nc.scalar.activation(
                            dst2[:, mt, :], ps[:], A.Identity,
                            bias=bvec[:, mt : mt + 1],
                        )

                # batched p_attn softmax: scores are ~N(0,1) so exp
                # needs no max shift (masked entries exp(-1e9) -> 0)
                inv_sqrt = 1.0 / math.sqrt(DK)
                scw = tmpQ.tile([128, B * H, 128], F32, name="scw")
                for b in range(B):
                    for h in range(H):
                        ps = psQ.tile([128, 128], F32, name="psQ")
                        c0 = h * DK
                        segs = []
                        base = c0
                        while base < c0 + DK:
                            s_i, p0 = base // 128, base % 128
                            n = min(128 - p0, c0 + DK - base)
                            segs.append((s_i, p0, n))
                            base += n
                        for si, (s_i, p0, n) in enumerate(segs):
                            nc.tensor.matmul(
                                ps[:],
                                qpT[p0 : p0 + n, s_i, b * L : (b + 1) * L],
                                kpT[p0 : p0 + n, s_i, b * L : (b + 1) * L],
                                start=(si == 0),
                                stop=(si == len(segs) - 1),
                            )
                        nc.vector.scalar_tensor_tensor(
                            scw[:, b * H + h, :], ps[:], inv_sqrt,
                            pm_sb[:, b, :], Alu.mult, Alu.add,
                        )
                ew = tmpQ.tile([128, B * H, 128], F32, name="ew")
                nc.scalar.activation(
                    ew[:].rearrange("p q z -> p (q z)"),
                    scw[:].rearrange("p q z -> p (q z)"), A.Exp,
                )
                esw = tmpQ.tile([128, B * H], F32, name="esw")
                nc.vector.tensor_reduce(esw[:], ew[:], Ax.X, Alu.add)
                rcw = tmpQ.tile([128, B * H], F32, name="rcw")
                nc.vector.reciprocal(rcw[:], esw[:])
                nc.vector.tensor_tensor(
                    pattn[:], ew[:],
                    rcw[:][:, :, None].broadcast_to([128, B * H, 128]),
                    Alu.mult,
                )

            # ============ stage G: batched final combine (replicated) ============
            # comb[p=i, b, h, j]:  i = (x%32)*4 + y//32,  j = (y%32)*4 + r,
            # h = x//32.  Partition p for core c covers (c%2)*64 + xl*4 + y2.
            with (
                tc.tile_pool(name="finp", bufs=2) as finp,
                tc.tile_pool(name="psF", bufs=2, space="PSUM") as psF,
            ):
                comb = finp.tile([128, B, H, 128], F32, name="comb")
                nc.vector.tensor_copy(
                    comb[:].rearrange("p b h z -> p (b h) z"), pattn[:]
                )
                # scg[p, b, h, r, y1] via 8 DMAs per r
                scg = finp.tile([128, B, H, 2, 32], F32, name="scg")
                for r in range(NC):
                    for c in range(CORES):
                        p0 = (c % 2) * 64
                        nc.sync.dma_start(
                            scg[p0 : p0 + 64, :, c // 2, r, :],
                            ag_out[r][c].rearrange(
                                "x y2 b y1 -> (x y2) b y1"
                            ),
                        )
                comb_v = comb[:].rearrange("p b h (y1 r4) -> p b h y1 r4", r4=4)
                for r in range(NC):
                    nc.vector.tensor_tensor(
                        comb_v[:, :, :, :, r], comb_v[:, :, :, :, r],
                        scg[:, :, :, r, :], Alu.add,
                    )
                # global min/max -> 1/(hi-lo) broadcast to all partitions
                mm = finp.tile([128, 2], F32, name="mm")
                cflat = comb[:].rearrange("p b h z -> p (b h z)")
                nc.vector.tensor_reduce(mm[:, 0:1], cflat, Ax.X, Alu.max)
                nc.vector.tensor_reduce(
                    mm[:, 1:2], cflat, Ax.X, Alu.min, negate=True
                )
                psf = psF.tile([128, 128], F32, name="psF")
                nc.tensor.transpose(psf[0:2, :], mm[:], ident32[:])
                hilo = finp.tile([2, 128], F32, name="hilo")
                nc.vector.tensor_copy(hilo[:], psf[0:2, :])
                hl2 = finp.tile([2, 1], F32, name="hl2")
                nc.vector.tensor_reduce(hl2[:], hilo[:], Ax.X, Alu.max)
                hl16 = finp.tile([2, 1], F16, name="hl16")
                nc.vector.tensor_copy(hl16[:], hl2[:])
                ones2 = finp.tile([2, 128], F16, name="ones2")
                nc.sync.dma_start(
                    ones2[:], ones_in[0:1, 0:128].to_broadcast([2, 128])
                )
                psr = psF.tile([128, 1], F32, name="psr")
                nc.tensor.matmul(psr[:], ones2[:], hl16[:], start=True, stop=True)
                rcpb = finp.tile([128, 1], F32, name="rcpb")
                nc.vector.reciprocal(rcpb[:], psr[:])

                # softmax over j per pane; lo/(hi-lo) shift is a row constant
                # -> softmax-invariant; comb*rcp is bounded so exp needs no
                # max shift
                nrm = finp.tile([128, B * H * 128], F32, name="nrm")
                nc.vector.tensor_scalar_mul(nrm[:], cflat, rcpb[:])
                ef = finp.tile([128, B * H, 128], F32, name="ef")
                nc.scalar.activation(
                    ef[:].rearrange("p q z -> p (q z)"), nrm[:], A.Exp
                )
                esums = finp.tile([128, B * H], F32, name="esums")
                nc.vector.tensor_reduce(esums[:], ef[:], Ax.X, Alu.add)
                recs = finp.tile([128, B * H], F32, name="recs")
                nc.vector.reciprocal(recs[:], esums[:])
                of = finp.tile([128, B * H, 128], F32, name="of")
                nc.vector.tensor_tensor(
                    of[:], ef[:],
                    recs[:][:, :, None].broadcast_to([128, B * H, 128]),
                    Alu.mult,
                )
                nc.sync.dma_start(
                    out[:].rearrange("b h i j -> i (b h) j"), of[:]
                )

    _split_multiwaits(nc)
    return nc


# ----------------------------------------------------------------------------
# Host-side input preparation (per core)
# ----------------------------------------------------------------------------


def prep_inputs(inputs):
    """inputs: dict of full numpy arrays as produced by setup_inputs().
    Returns (in_maps, npos): per-core input dicts + per-r positive counts."""
    f32 = np.float32
    f16 = np.float16
    te = np.ascontiguousarray(inputs["text_embeddings"], dtype=f32)  # [B, L, D]
    query = np.ascontiguousarray(inputs["query"], dtype=f32)
    key = np.ascontiguousarray(inputs["key"], dtype=f32)
    mask = inputs["mask"]
    Wtri = np.ascontiguousarray(inputs["Wtri"], dtype=f32)  # [385, 384, 385, 2]
    Vw = inputs["Vw"].astype(f32).reshape(2, D)

    def kt(a, s):  # [K, M] -> [128, K//128, M]
        K, M = a.shape
        assert K == s * 128
        return np.ascontiguousarray(a.reshape(s, 128, M).transpose(1, 0, 2))

    def rowsT(a):  # [B, L, D] -> [128, D//128, B*L] transposed k-tiled
        Dm = a.shape[-1]
        flat = a.reshape(-1, Dm).T  # [D, B*L]
        return np.ascontiguousarray(
            flat.reshape(Dm // 128, 128, flat.shape[1]).transpose(1, 0, 2)
        )

    def bias_t(b, s):  # [s*128] -> [128, s]
        return np.ascontiguousarray(b.reshape(s, 128).T)

    # per-r sign-sorted |Vw|-scaled text embeddings: ter[z, b, r, h']
    te_zbd = te.transpose(1, 0, 2)  # [L(z), B, D]
    ter = np.empty((L, B, NC, D), f32)
    npos = []
    for r in range(NC):
        order = np.argsort((Vw[r] <= 0).astype(np.int32), kind="stable")
        npos.append(int((Vw[r] > 0).sum()))
        ter[:, :, r, :] = te_zbd[:, :, order] * np.abs(Vw[r][order])
    npos = tuple(npos)

    common = {
        "teT": rowsT(te).astype(f16),
        "ter": np.ascontiguousarray(ter).astype(f16),
        "Wq": kt(inputs["Wq"].astype(f32), 6).astype(f16),
        "bq": bias_t(inputs["bq"].astype(f32), 6),
        "Wk": kt(inputs["Wk"].astype(f32), 6).astype(f16),
        "bk": bias_t(inputs["bk"].astype(f32), 6),
        "qryT": rowsT(query).astype(f16),
        "keyT": rowsT(key).astype(f16),
        "pmask": np.ascontiguousarray(
            np.where(mask == 0, np.float32(-1e9), np.float32(0.0)).transpose(1, 0, 2)
        ),
        "Vb": inputs["Vb"].astype(f32).reshape(2, 1),
        "ones16": np.ones((1, BL), f16),
    }
    for nm in ("h", "m", "t"):
        common[f"W{nm}1"] = kt(inputs[f"W{nm}1"].astype(f32), 6).astype(f16)
        common[f"b{nm}1"] = bias_t(inputs[f"b{nm}1"].astype(f32), 3)
        common[f"W{nm}2"] = kt(inputs[f"W{nm}2"].astype(f32), 3).astype(f16)
        common[f"b{nm}2"] = bias_t(inputs[f"b{nm}2"].astype(f32), 3)

    # T-correction weights: WT[j, k, r] = Wtri[384, k, j, r]
    WT = np.ascontiguousarray(Wtri[384].transpose(1, 0, 2))  # [385, 384, 2]
    common["WbiT"] = np.ascontiguousarray(
        WT[:HD].reshape(3, 128, HD, 2).transpose(1, 0, 3, 2)
    ).astype(np.float16)  # [128, 3, 2, 384]
    common["Wlast"] = np.ascontiguousarray(
        WT[HD].T.reshape(1, 2, HD)
    ).astype(np.float16)

    idx = np.arange(L)
    in_maps = []
    for c in range(CORES):
        m = dict(common)
        j0 = c * JC
        # W core slice; jr rows ordered [e2_r0, e2_r1, r0 jj0..47, r1 jj0..47]
        blk = Wtri[:HD, :, j0 : j0 + JC, :]  # [384 i, 384 k, 48 j, 2 r]
        w_rows = np.ascontiguousarray(
            blk.transpose(3, 2, 1, 0).reshape(NJR, 3, 128, HD)
        )  # [row, ks, k_p, i], r-major
        bj = Wtri[:HD, :, 384, :]  # [384 i, 384 k, 2 r]
        w0 = np.empty((2, 3, 128, HD), dtype=f32)
        for r in range(2):
            w0[r] = bj[:, :, r].T.reshape(3, 128, HD)
        # W1c0: [128 k_p, 3 ks, 2 jl, HD i]
        m["W1c0"] = np.ascontiguousarray(w0.transpose(2, 1, 0, 3)).astype(f16)
        # W1c: [ch, 128 k_p, 3 ks, 4 jl, HD i]
        wc = w_rows.reshape(NCH, W_CHUNK, 3, 128, HD)
        m["W1c"] = np.ascontiguousarray(wc.transpose(0, 3, 2, 1, 4)).astype(f16)

        # softmax-z additive masks for this core's x chunk: [y(128), xl, z]
        xs = c * XL + np.arange(XL)
        zz = idx[None, None, :]
        yy = idx[:, None, None]
        xx = xs[None, :, None]
        bad = (zz > yy) | (zz < xx)  # [y, xl, z]
        m["madd"] = np.ascontiguousarray(np.where(bad, -1e6, 1e6).astype(f32))
        in_maps.append(m)
    return in_maps, npos


_CACHE = {}


def _get_built(npos, debug=False):
    key = ("nc", npos, debug)
    if key not in _CACHE:
        _CACHE[key] = build(npos, debug=debug)
    return _CACHE[key]


def run(inputs, debug=False, trace=False):
    from concourse.bass_utils import run_bass_kernel_spmd

    in_maps, npos = prep_inputs(inputs)
    nc = _get_built(npos, debug=debug)
    res = run_bass_kernel_spmd(
        nc, in_maps, list(range(CORES)), trace=trace
    )
    return res


def kernel(**inputs):
    res = run(inputs, debug=False)
    return np.ascontiguousarray(res.results[0]["out"])


if __name__ == "__main__":
    nc = build((384, 384), debug=False)
    print("build OK")


# revision 30
# speedup vs baseline: 1.0270x; 1.0270x over previous
"""Trainium2 Bass kernel for nn_MultiHeadAttention_88854283419963 (TriAffine attention).

8 NeuronCores, SPMD.  The TriAffine contraction
    s[b,x,y,z,r] = sum_{i,k,j} xaug[b,x,i] mid[b,z,k] Wtri[i,k,j,r] yaug[b,y,j]
is factored k -> i -> j.  Wtri is sharded along j (48 j's per core).

v8 (v6 538us -> v7 458us -> this):
  - W streamed with ONE contiguous DMA per chunk (host pre-tiled), 5-chunk
    prefetch so MM1 rides through HBM contention from concurrent AllToAlls.
  - u staged to the AllToAll inputs incrementally from a small rotating
    buffer (dest-major x-minor == partition order -> linear [128,..] DMA).
    FOUR pipelined AllToAlls triggered after chunks 6/12/18/24, so the
    collective stream starts at ~35% of stage C and the last (quarter-
    size) piece is the only exposed tail.
  - All MLPs (incl. tail) + the A_r chain run in stage A with weights
    loaded upfront -- v7 lost ~50us loading tail-MLP weights through
    a2a-saturated HBM at stage-C end.
  - Stage E/F: the t_bias/corner correction T[y,z] is folded into MM3 as
    3 extra accumulating matmuls (lhsT=A_r, rhs=mid broadcast over x), so
    softmax reads PSUM directly; Vw is folded into a per-r sign-sorted,
    |Vw|-scaled copy of text_embeddings so the post-MM4 reduction is two
    relu-accumulates straight from PSUM (relu(S*|w|)=|w|*relu(S)), the
    [128,768] relu+dot chain of v6/v7 is gone.  The transpose scratch
    lives in ps4's spare columns (no extra PSUM bank).
  - q/k/p_attn fills the a2a-tail bubble between the two E/F halves.
  - Stage G runs all 16 (b,h) panes in [128,2048]-wide batched ops.
"""

import sys

sys.path.insert(0, "/opt/trn_rl_repo")
sys.path.insert(0, "/root/.axon_site/_ro/trn_rl_repo")

import math

import numpy as np

import concourse.bass as bass
import concourse.mybir as mybir
from concourse.masks import make_identity
from concourse.tile import TileContext
from bass_rust import ScopedClock

# ----------------------------------------------------------------------------
# Workaround: this container's walrus build rejects >1 sync-wait on the CTRL
# (Drain) instruction Tile emits at the kernel tail ("Too many sync wait
# commands").  Split the waits across single-wait NOPs instead.
# ----------------------------------------------------------------------------


def _patched_drain_and_barrier(self, tick_clock, wait_clock):
    probe = self.nc.sync.nop()
    wait_clock.add_sem_waits(probe.ins, ScopedClock({None: tick_clock.global_clock}))
    si = probe.ins.sync_info
    if si is not None and len(si.on_wait) > 1:
        waits = list(si.on_wait)
        probe.ins.sync_info = mybir.SyncInfo(
            on_wait=[waits[0]], on_update=list(si.on_update)
        )
        for w in waits[1:]:
            extra = self.nc.sync.nop()
            extra.ins.sync_info = mybir.SyncInfo(on_wait=[w], on_update=[])
    self.nc.sync.drain()
    self.nc.all_engine_barrier()
    assert self.sems is not None
    popped = self.nc._tile_sem_poison_stack.pop()
    assert popped is self._sem_poison
    self.nc.clear_and_free_semaphores(list(self.sems.allocated().values()))
    self.nc.all_engine_barrier()


TileContext._drain_and_barrier = _patched_drain_and_barrier

_NOPN = [0]


def _split_multiwaits(nc, limit=1):
    """walrus in this container accepts at most one sync-wait per instruction;
    move extra waits onto same-engine NoOps inserted just before."""
    for f in nc.m.functions:
        for blk in f.blocks:
            changed = False
            new = []
            for inst in blk.instructions:
                si = getattr(inst, "sync_info", None)
                if si is not None and len(si.on_wait) > limit:
                    ow = list(si.on_wait)
                    for w in ow[:-limit]:
                        _NOPN[0] += 1
                        nop = mybir.InstNoOp(name=f"mwsplit_{_NOPN[0]}", ins=[], outs=[])
                        nop.engine = inst.engine
                        nop.sync_info = mybir.SyncInfo(on_wait=[w], on_update=[])
                        new.append(nop)
                    inst.sync_info = mybir.SyncInfo(
                        on_wait=ow[-limit:], on_update=list(si.on_update)
                    )
                    changed = True
                new.append(inst)
            if changed:
                blk.instructions = new

# ----------------------------------------------------------------------------
B, L, D = 2, 128, 768
H, DK = 4, 192
HD, NC = 384, 2
CORES = 8
JC = HD // CORES          # 48
XL = L // CORES           # 16
NJR = 2 * JC              # 96
BL = B * L                # 256

F32 = mybir.dt.float32
F16 = mybir.dt.float16

DT_STORE = F16
W_CHUNK = 4               # jr's per streamed W chunk
NCH = NJR // W_CHUNK      # 24 full chunks after the uex chunk

# AllToAll pieces (trigger after these chunks):
# chunk 0 = uex (2 rows); chunks 1..12 = r0 jj 0..47; 13..24 = r1 jj 0..47
PIECES = [
    (4, 2 + 12),   # uex + r0 jj 0..11  (small: starts the wire early)
    (12, 36),      # r0 jj 12..47
    (18, 24),      # r1 jj 0..23
    (24, 24),      # r1 jj 24..47
]

A = mybir.ActivationFunctionType
Alu = mybir.AluOpType
Ax = mybir.AxisListType


def build(npos, debug=False):
    """npos: (npos_r0, npos_r1) -- column counts of the positive-|Vw| halves
    of the per-r sign-sorted text embeddings."""
    nc = bass.Bass(num_devices=CORES)

    # ---- inputs ----
    teT = nc.dram_tensor("teT", [128, 6, BL], DT_STORE, kind="ExternalInput")
    # per-r sign-sorted, |Vw|-scaled text embeddings: [z, b, r, h']
    terd = nc.dram_tensor("ter", [128, B, NC, D], DT_STORE, kind="ExternalInput")
    mlp_in = {}
    for nm in ("h", "m", "t"):
        mlp_in[nm] = (
            nc.dram_tensor(f"W{nm}1", [128, 6, HD], DT_STORE, kind="ExternalInput"),
            nc.dram_tensor(f"b{nm}1", [128, 3], F32, kind="ExternalInput"),
            nc.dram_tensor(f"W{nm}2", [128, 3, HD], DT_STORE, kind="ExternalInput"),
            nc.dram_tensor(f"b{nm}2", [128, 3], F32, kind="ExternalInput"),
        )

    Wq = nc.dram_tensor("Wq", [128, 6, D], DT_STORE, kind="ExternalInput")
    bq = nc.dram_tensor("bq", [128, 6], F32, kind="ExternalInput")
    Wk = nc.dram_tensor("Wk", [128, 6, D], DT_STORE, kind="ExternalInput")
    bk = nc.dram_tensor("bk", [128, 6], F32, kind="ExternalInput")
    qryT = nc.dram_tensor("qryT", [128, 6, BL], DT_STORE, kind="ExternalInput")
    keyT = nc.dram_tensor("keyT", [128, 6, BL], DT_STORE, kind="ExternalInput")
    pmask = nc.dram_tensor("pmask", [128, B, L], F32, kind="ExternalInput")

    # W chunks, host-tiled: one contiguous [128, 3*g*HD] DMA each
    W1c0 = nc.dram_tensor("W1c0", [128, 3, 2, HD], DT_STORE, kind="ExternalInput")
    W1c = nc.dram_tensor("W1c", [NCH, 128, 3, W_CHUNK, HD], DT_STORE,
                         kind="ExternalInput")
    WbiT = nc.dram_tensor("WbiT", [128, 3, 2, HD], DT_STORE, kind="ExternalInput")
    Wlast = nc.dram_tensor("Wlast", [1, 2, HD], DT_STORE, kind="ExternalInput")
    ones_in = nc.dram_tensor("ones16", [1, BL], DT_STORE, kind="ExternalInput")

    madd = nc.dram_tensor("madd", [128, XL, 128], F32, kind="ExternalInput")
    Vb_in = nc.dram_tensor("Vb", [2, 1], F32, kind="ExternalInput")

    out = nc.dram_tensor("out", [B, H, L, L], F32, kind="ExternalOutput")

    # a2a pieces.  Inputs are declared [(dest x)=128, b, rows, z]:
    # dest-major x-minor matches the partition order of the u tiles, so
    # staging is a linear [128,...] DMA and the collective's linear
    # dest-chunking still sees [dest][16x, b, rows, z].
    a2a_in, a2a_out = [], []
    for pi, (_, rows) in enumerate(PIECES):
        a2a_in.append(
            nc.dram_tensor(f"a2a{pi}_in", [128, B, rows, 128], DT_STORE)
        )
        a2a_out.append(
            nc.dram_tensor(f"a2a{pi}_out", [CORES, XL, B, rows, 128], DT_STORE)
        )
    # score AG payload per r: [xl, y2, b, y1]  (y = y2*32 + y1)
    ag_in = [nc.dram_tensor(f"ag_in{r}", [XL, 4, B, 32], F32) for r in range(NC)]
    ag_out = [
        nc.dram_tensor(f"ag_out{r}", [CORES, XL, 4, B, 32], F32,
                       addr_space="Shared")
        for r in range(NC)
    ]

    # (tile_idx, p0, src_core, jj0, n): partition runs for R3 gathers with a
    # restricted jj range [lo, hi) per source core
    def r3_slices(lo, hi):
        runs = []
        for s in range(CORES):
            jj0 = lo
            while jj0 < hi:
                gl = s * JC + jj0
                t, p0 = gl // 128, gl % 128
                n = min(128 - p0, hi - jj0)
                runs.append((t, p0, s, jj0, n))
                jj0 += n
        return runs

    with TileContext(nc) as tc:
        with (
            tc.tile_pool(name="res", bufs=1) as res,
            tc.tile_pool(name="res16", bufs=1) as res16,
        ):
            ident32 = res.tile([128, 128], F32)
            make_identity(nc, ident32)

            headT16 = res16.tile([128, 3, BL], DT_STORE, name="headT16")
            midT16 = res16.tile([128, 3, BL], DT_STORE, name="midT16")
            tailT16 = res16.tile([128, 3, BL], DT_STORE, name="tailT16")
            pattn = res.tile([128, B * H, L], F32, name="pattn")
            score_r = [
                res.tile([128, B * XL], F32, name=f"score_r{r}") for r in range(NC)
            ]
            A_sb = res16.tile([128, 3, 2, BL], DT_STORE, name="A_sb")
            ones_sb = res16.tile([1, BL], DT_STORE, name="ones16")
            ter_sb = res16.tile([128, B, NC, D], DT_STORE, name="ter")
            vbb = res.tile([128, 2], F32)
            madd_sb = res.tile([128, XL, 128], F32)
            ones_col = res.tile([128, 1], F32, name="ones_col")

            # R3 (j-on-partition u tiles) + uex rows live into stage E/F
            R3_r0, R3_r1, uexrows = {}, {}, {}
            for b in range(B):
                R3_r0[b] = res16.tile([128, 3, XL * 128], DT_STORE, name=f"R3a{b}")
                R3_r1[b] = res16.tile([128, 3, XL * 128], DT_STORE, name=f"R3b{b}")
                for r in range(NC):
                    uexrows[(r, b)] = res16.tile(
                        [1, XL * 128], DT_STORE, name=f"uexr_{r}{b}"
                    )

            # ============ stage A: all MLPs + A_r chain ============
            with (
                tc.tile_pool(name="mlpw0", bufs=1) as mlpw0,
                tc.tile_pool(name="psA", bufs=2, space="PSUM") as psA,
                tc.tile_pool(name="tmpA", bufs=2) as tmpA,
            ):
                teT_sb = mlpw0.tile([128, 6, BL], DT_STORE, name="teT_sb")
                nc.sync.dma_start(teT_sb[:], teT[:])
                mlpw = {}
                for nm in ("m", "h", "t"):
                    W1d, b1d, W2d, b2d = mlp_in[nm]
                    eng = nc.scalar if nm == "t" else nc.sync
                    w1 = mlpw0.tile([128, 6, HD], DT_STORE, name=f"w1_{nm}")
                    eng.dma_start(w1[:], W1d[:])
                    w2 = mlpw0.tile([128, 3, HD], DT_STORE, name=f"w2_{nm}")
                    eng.dma_start(w2[:], W2d[:])
                    b1 = mlpw0.tile([128, 3], F32, name=f"b1_{nm}")
                    eng.dma_start(b1[:], b1d[:])
                    b2 = mlpw0.tile([128, 3], F32, name=f"b2_{nm}")
                    eng.dma_start(b2[:], b2d[:])
                    mlpw[nm] = (w1, b1, w2, b2)
                wbiT_sb = mlpw0.tile([128, 3, 2, HD], DT_STORE, name="wbiT")
                nc.scalar.dma_start(wbiT_sb[:], WbiT[:])
                wlast_sb = mlpw0.tile([1, 2, HD], DT_STORE, name="wlast")
                nc.scalar.dma_start(wlast_sb[:], Wlast[:])
                # small E/F-time loads (after the compute-critical ones)
                nc.scalar.dma_start(ter_sb[:], terd[:])
                for r in range(2):
                    nc.scalar.dma_start(
                        vbb[:, r : r + 1], Vb_in[r : r + 1, :].to_broadcast([128, 1])
                    )
                nc.scalar.dma_start(madd_sb[:], madd[:])
                nc.scalar.dma_start(ones_sb[:], ones_in[:])
                nc.vector.memset(ones_col[:], 1.0)

                for nm, dst in (("m", midT16), ("h", headT16), ("t", tailT16)):
                    w1, b1, w2, b2 = mlpw[nm]
                    h1 = tmpA.tile([128, 3, BL], DT_STORE, name="h1")
                    for mt in range(3):
                        ps = psA.tile([128, BL], F32, name="psA")
                        for ks in range(6):
                            nc.tensor.matmul(
                                ps[:], w1[:, ks, mt * 128 : (mt + 1) * 128],
                                teT_sb[:, ks, :], start=(ks == 0), stop=(ks == 5),
                            )
                        nc.scalar.activation(
                            h1[:, mt, :], ps[:], A.Relu, bias=b1[:, mt : mt + 1]
                        )
                    for mt in range(3):
                        ps = psA.tile([128, BL], F32, name="psA")
                        for ks in range(3):
                            nc.tensor.matmul(
                                ps[:], w2[:, ks, mt * 128 : (mt + 1) * 128],
                                h1[:, ks, :], start=(ks == 0), stop=(ks == 2),
                            )
                        nc.scalar.activation(
                            dst[:, mt, :], ps[:], A.Identity, bias=b2[:, mt : mt + 1]
                        )

                # A_r[k,y] = sum_j WbiT[j,k,r] tail_aug[j,y]  (T-fold lhsT)
                for r in range(2):
                    for kt in range(3):
                        ps = psA.tile([128, BL], F32, name="psA")
                        for jt in range(3):
                            nc.tensor.matmul(
                                ps[:], wbiT_sb[:, jt, r, kt * 128 : (kt + 1) * 128],
                                tailT16[:, jt, :], start=(jt == 0), stop=False,
                            )
                        nc.tensor.matmul(
                            ps[:], wlast_sb[:, r, kt * 128 : (kt + 1) * 128],
                            ones_sb[:], start=False, stop=True,
                        )
                        nc.scalar.activation(A_sb[:, kt, r, :], ps[:], A.Copy)

            # ================= stage C: chunk loop (MM1 + MM2) =================
            # piece-relative row offset for chunk ch (ch0 = 2 uex rows,
            # ch k>=1 = rows 2+(k-1)*4 .. +4 in global [uex|r0|r1] order)
            piece_start = [0]
            for _, rows in PIECES:
                piece_start.append(piece_start[-1] + rows)

            with (
                tc.tile_pool(name="urot", bufs=8) as urot,
                tc.tile_pool(name="wchunk", bufs=5) as wchunk,
                tc.tile_pool(name="tbig", bufs=2) as tbigp,
                tc.tile_pool(name="psU", bufs=2, space="PSUM") as psU,
                tc.tile_pool(name="psT", bufs=2, space="PSUM") as psT,
            ):
                for ch in range(NCH + 1):
                    wt = wchunk.tile([128, 3, W_CHUNK, HD], DT_STORE, name="wt")
                    if ch == 0:
                        g = 2
                        nc.sync.dma_start(wt[:, :, :2, :], W1c0[:])
                    else:
                        g = W_CHUNK
                        nc.sync.dma_start(wt[:], W1c[ch - 1])

                    # MM1: t_big[i, it, jl, (b z)]
                    t_big = tbigp.tile([128, 3, W_CHUNK, BL], DT_STORE, name="t_big")
                    for jl in range(g):
                        ps = psT.tile([128, 3, BL], F32, name="psT")
                        for it in range(3):
                            for ks in range(3):
                                nc.tensor.matmul(
                                    ps[:, it, :],
                                    wt[:, ks, jl, it * 128 : (it + 1) * 128],
                                    midT16[:, ks, :],
                                    start=(ks == 0), stop=(ks == 2),
                                )
                        nc.vector.tensor_copy(t_big[:, :, jl, :], ps[:])

                    # MM2: u[x, (jl z)] per b into a small rotating buffer
                    # (its only reader is the staging DMA right below)
                    uch = urot.tile([128, B, W_CHUNK, 128], DT_STORE, name="uch")
                    for b in range(B):
                        psu = psU.tile([128, W_CHUNK * 128], F32, name="psU")
                        rhs_n = g * 128
                        for it in range(3):
                            nc.tensor.matmul(
                                psu[:, :rhs_n],
                                headT16[:, it, b * L : (b + 1) * L],
                                t_big[:, it, :g, b * L : (b + 1) * L],
                                start=(it == 0), stop=(it == 2),
                            )
                        nc.vector.tensor_copy(
                            uch[:, b, :g, :],
                            psu[:, : g * 128].rearrange("p (g z) -> p g z", z=128),
                        )

                    # ---- incremental a2a staging (pool queue) ----
                    grow = 0 if ch == 0 else 2 + (ch - 1) * W_CHUNK
                    pi = max(i for i in range(len(PIECES)) if piece_start[i] <= grow)
                    rr = grow - piece_start[pi]
                    nc.gpsimd.dma_start(
                        a2a_in[pi][:, :, rr : rr + g, :],
                        uch[:, :, :g, :],
                    )

                    # ---- piece triggers + receive-side gathers ----
                    for pi2, (tch, _) in enumerate(PIECES[:-1]):
                        if ch == tch:
                            nc.gpsimd.collective_compute(
                                "AllToAll", Alu.bypass,
                                replica_groups=[list(range(CORES))],
                                ins=[a2a_in[pi2][:]], outs=[a2a_out[pi2][:]],
                            )
                            # r0 gathers on scalar (idle during stage C)
                            if pi2 == 0:
                                for b in range(B):
                                    for r in range(NC):
                                        nc.scalar.dma_start(
                                            uexrows[(r, b)][:].rearrange(
                                                "c (x z) -> c x z", z=128
                                            ),
                                            a2a_out[0][0:1, :, b, r, :],
                                        )
                                for b in range(B):
                                    for t, p0, s, jj0, n in r3_slices(0, 12):
                                        nc.scalar.dma_start(
                                            R3_r0[b][p0 : p0 + n, t, :].rearrange(
                                                "p (x z) -> p x z", z=128
                                            ),
                                            a2a_out[0][
                                                s, :, b, 2 + jj0 : 2 + jj0 + n, :
                                            ].rearrange("x j z -> j x z"),
                                        )
                            if pi2 == 1:
                                for b in range(B):
                                    for t, p0, s, jj0, n in r3_slices(12, JC):
                                        nc.scalar.dma_start(
                                            R3_r0[b][p0 : p0 + n, t, :].rearrange(
                                                "p (x z) -> p x z", z=128
                                            ),
                                            a2a_out[1][
                                                s, :, b, jj0 - 12 : jj0 - 12 + n, :
                                            ].rearrange("x j z -> j x z"),
                                        )

                # last piece: trigger + r1 gathers (pool; nothing queued after)
                nc.gpsimd.collective_compute(
                    "AllToAll", Alu.bypass,
                    replica_groups=[list(range(CORES))],
                    ins=[a2a_in[3][:]], outs=[a2a_out[3][:]],
                )
                for b in range(B):
                    for t, p0, s, jj0, n in r3_slices(0, 24):
                        nc.sync.dma_start(
                            R3_r1[b][p0 : p0 + n, t, :].rearrange(
                                "p (x z) -> p x z", z=128
                            ),
                            a2a_out[2][
                                s, :, b, jj0 : jj0 + n, :
                            ].rearrange("x j z -> j x z"),
                        )
                for b in range(B):
                    for t, p0, s, jj0, n in r3_slices(24, JC):
                        nc.sync.dma_start(
                            R3_r1[b][p0 : p0 + n, t, :].rearrange(
                                "p (x z) -> p x z", z=128
                            ),
                            a2a_out[3][
                                s, :, b, jj0 - 24 : jj0 - 24 + n, :
                            ].rearrange("x j z -> j x z"),
                        )

            # ============ stage E/F: MM3(+T) + softmax-z + MM4 + score ============
            def ef_half(r, postp, post16, ps4p, psSp):
                accP = postp.tile([128, B * XL], F32, name="accP")
                accN = postp.tile([128, B * XL], F32, name="accN")
                esum_all = postp.tile([128, B * XL], F32, name="esumA")
                for b in range(B):
                    R3 = R3_r0[b] if r == 0 else R3_r1[b]
                    uexrow = uexrows[(r, b)]
                    for hh in range(2):
                        h0 = hh * 1024
                        psS = psSp.tile([128, 1024], F32, name="psS")
                        for q in range(2):
                            c0 = h0 + q * 512
                            l0 = q * 512
                            for t in range(3):
                                nc.tensor.matmul(
                                    psS[:, l0 : l0 + 512],
                                    tailT16[:, t, b * L : (b + 1) * L],
                                    R3[:, t, c0 : c0 + 512],
                                    start=(t == 0), stop=False,
                                )
                            # T-fold: psS[y, xz] += sum_k A_r[k,y] mid[z,k]
                            # (mid broadcast over the 4 x's of this segment)
                            for kt in range(3):
                                nc.tensor.matmul(
                                    psS[:, l0 : l0 + 512].rearrange(
                                        "p (x z) -> p x z", z=128
                                    ),
                                    A_sb[:, kt, r, b * L : (b + 1) * L],
                                    midT16[:, kt, b * L : (b + 1) * L][
                                        :, None, :
                                    ].broadcast_to([128, 4, 128]),
                                    start=False, stop=False,
                                )
                            nc.tensor.matmul(
                                psS[:, l0 : l0 + 512], ones_sb[:, 0:128],
                                uexrow[:, c0 : c0 + 512],
                                start=False, stop=True,
                            )
                        for xl0 in range(hh * 8, hh * 8 + 8, 2):
                          c0 = (xl0 - hh * 8) * 128
                          # clip-mask reads PSUM directly; min(s, +-1e6) gives
                          # EXACTLY -1e6 on masked entries
                          sm2 = postp.tile([128, 2, 128], F32, name="sm2")
                          nc.vector.tensor_tensor(
                              sm2[:],
                              psS[:, c0 : c0 + 256].rearrange(
                                  "p (u z) -> p u z", z=128
                              ),
                              madd_sb[:, xl0 : xl0 + 2, :], Alu.min,
                          )
                          # per-xl max bias: fully-masked rows (y < x,
                          # all entries -1e6) must softmax to exactly
                          # uniform, so the bias cannot come from the
                          # sibling xl
                          mx2 = postp.tile([128, 2], F32, name="mx2")
                          nc.vector.tensor_reduce(
                              mx2[:], sm2[:], Ax.X, Alu.max, negate=True
                          )
                          e2 = postp.tile([128, 2, 128], F32, name="e2")
                          for u_i in range(2):
                              nc.scalar.activation(
                                  e2[:, u_i, :], sm2[:, u_i, :],
                                  A.Exp, bias=mx2[:, u_i : u_i + 1],
                              )
                          nc.vector.tensor_reduce(
                              esum_all[:, b * XL + xl0 : b * XL + xl0 + 2],
                              e2[:], Ax.X, Alu.add,
                          )
                          for xl in (xl0, xl0 + 1):
                            u_i = xl - xl0
                            # ps4 cols 768:896 double as transpose scratch
                            ps4 = ps4p.tile([128, 1024], F32, name="ps4")
                            nc.tensor.transpose(
                                ps4[:, 768:896], e2[:, u_i, :], ident32[:]
                            )
                            eT = post16.tile([128, 128], DT_STORE, name="eT")
                            if xl % 2 == 0:
                                nc.scalar.activation(eT[:], ps4[:, 768:896], A.Copy)
                            else:
                                nc.vector.tensor_copy(eT[:], ps4[:, 768:896])
                            nc.tensor.matmul(
                                ps4[:, 0:512], eT[:], ter_sb[:, b, r, 0:512],
                                start=True, stop=True,
                            )
                            nc.tensor.matmul(
                                ps4[:, 512:768], eT[:], ter_sb[:, b, r, 512:768],
                                start=True, stop=True,
                            )
                            # sign-trick: score = (sum_pos relu - sum_neg relu)
                            # / esum + Vb, straight from PSUM
                            npos_r = npos[r]
                            col = b * XL + xl
                            jkp = post16.tile([128, D], DT_STORE, name="jkp")
                            nc.scalar.activation(
                                jkp[:, 0:npos_r],
                                ps4[:, 0:npos_r], A.Relu,
                                accum_out=accP[:, col : col + 1],
                            )
                            jkn = post16.tile([128, D], DT_STORE, name="jkn")
                            nc.vector.scalar_tensor_tensor(
                                jkn[:, 0 : D - npos_r],
                                ps4[:, npos_r:D], 0.0,
                                ones_col[:, 0:1].broadcast_to(
                                    [128, D - npos_r]
                                ),
                                Alu.max, Alu.mult,
                                accum_out=accN[:, col : col + 1],
                            )

                # batched score finalization: 4 wide ops for all 32 cols
                dif = postp.tile([128, B * XL], F32, name="dif")
                nc.vector.tensor_tensor(dif[:], accP[:], accN[:], Alu.subtract)
                recA = postp.tile([128, B * XL], F32, name="recA")
                nc.vector.reciprocal(recA[:], esum_all[:])
                nc.vector.tensor_tensor(dif[:], dif[:], recA[:], Alu.mult)
                nc.vector.tensor_scalar(
                    score_r[r][:], dif[:], vbb[:, r : r + 1], None, Alu.add
                )

                # fire this r-half's score AllGather as soon as done.
                # payload [xl, y2, b, y1]: score cols (b xl) -> partitions
                # via PE transpose, then 2 DMAs scatter y into (y2, y1)
                psS = psSp.tile([128, 1024], F32, name="psS")
                nc.tensor.transpose(
                    psS[0 : B * XL, 0:128], score_r[r][:], ident32[:],
                )
                sc_t = postp.tile([B * XL, 128], F32, name="sc_t")
                nc.vector.tensor_copy(sc_t[:], psS[0 : B * XL, 0:128])
                for b in range(B):
                    nc.sync.dma_start(
                        ag_in[r][:, :, b, :],
                        sc_t[b * XL : (b + 1) * XL, :].rearrange(
                            "x (y2 y1) -> x y2 y1", y1=32
                        ),
                    )
                nc.gpsimd.collective_compute(
                    "AllGather", Alu.bypass,
                    replica_groups=[list(range(CORES))],
                    ins=[ag_in[r][:]], outs=[ag_out[r][:]],
                )

            with (
                tc.tile_pool(name="postp0", bufs=4) as postp,
                tc.tile_pool(name="post160", bufs=2) as post16,
                tc.tile_pool(name="ps40", bufs=3, space="PSUM") as ps4p,
                tc.tile_pool(name="psS0", bufs=1, space="PSUM") as psSp,
            ):
                ef_half(0, postp, post16, ps4p, psSp)

            with (
                tc.tile_pool(name="postp1", bufs=4) as postp,
                tc.tile_pool(name="post161", bufs=2) as post16,
                tc.tile_pool(name="ps41", bufs=3, space="PSUM") as ps4p,
                tc.tile_pool(name="psS1", bufs=1, space="PSUM") as psSp,
            ):
                ef_half(1, postp, post16, ps4p, psSp)

            # ---- q/k + p_attn: overlaps the score AllGathers / stage-G waits ----
            with (
                tc.tile_pool(name="qkw", bufs=1) as qkw,
                tc.tile_pool(name="psQ", bufs=2, space="PSUM") as psQ,
                tc.tile_pool(name="tmpQ", bufs=2) as tmpQ,
            ):
                wq_sb = qkw.tile([128, 6, D], DT_STORE, name="wqk")
                nc.sync.dma_start(wq_sb[:], Wq[:])
                bq_sb = qkw.tile([128, 6], F32, name="bqs")
                nc.sync.dma_start(bq_sb[:], bq[:])
                wk_sb = qkw.tile([128, 6, D], DT_STORE, name="wqk")
                nc.sync.dma_start(wk_sb[:], Wk[:])
                bk_sb = qkw.tile([128, 6], F32, name="bks")
                nc.sync.dma_start(bk_sb[:], bk[:])
                qT_sb = qkw.tile([128, 6, BL], DT_STORE, name="qkT")
                nc.sync.dma_start(qT_sb[:], qryT[:])
                kT_sb = qkw.tile([128, 6, BL], DT_STORE, name="qkT")
                nc.sync.dma_start(kT_sb[:], keyT[:])
                pm_sb = qkw.tile([128, B, L], F32, name="pm")
                nc.sync.dma_start(pm_sb[:], pmask[:])

                qpT = qkw.tile([128, 6, BL], DT_STORE, name="qpT")
                kpT = qkw.tile([128, 6, BL], DT_STORE, name="kpT")
                for wmat, bvec, src, dst2 in (
                    (wq_sb, bq_sb, qT_sb, qpT),
                    (wk_sb, bk_sb, kT_sb, kpT),
                ):
                    for mt in range(6):
                        ps = psQ.tile([128, BL], F32, name="psQ")
                        for ks in range(6):
                            nc.tensor.matmul(
                                ps[:],
                                wmat[:, ks, mt * 128 : (mt + 1) * 128],
                                src[:, ks, :],
                                start=(ks == 0), stop=(ks == 5),
                            )
                        nc.scalar.activation(
                            dst2[:, mt, :], ps[:], A.Identity,
                            bias=bvec[:, mt : mt + 1],
                        )

                # batched p_attn softmax: scores are ~N(0,1) so exp
                # needs no max shift (masked entries exp(-1e9) -> 0)
                inv_sqrt = 1.0 / math.sqrt(DK)
                scw = tmpQ.tile([128, B * H, 128], F32, name="scw")
                for b in range(B):
                    for h in range(H):
                        ps = psQ.tile([128, 128], F32, name="psQ")
                        c0 = h * DK
                        segs = []
                        base = c0
                        while base < c0 + DK:
                            s_i, p0 = base // 128, base % 128
                            n = min(128 - p0, c0 + DK - base)
                            segs.append((s_i, p0, n))
                            base += n
                        for si, (s_i, p0, n) in enumerate(segs):
                            nc.tensor.matmul(
                                ps[:],
                                qpT[p0 : p0 + n, s_i, b * L : (b + 1) * L],
                                kpT[p0 : p0 + n, s_i, b * L : (b + 1) * L],
                                start=(si == 0),
                                stop=(si == len(segs) - 1),
                            )
                        nc.vector.scalar_tensor_tensor(
                            scw[:, b * H + h, :], ps[:], inv_sqrt,
                            pm_sb[:, b, :], Alu.mult, Alu.add,
                        )
                ew = tmpQ.tile([128, B * H, 128], F32, name="ew")
                nc.scalar.activation(
                    ew[:].rearrange("p q z -> p (q z)"),
                    scw[:].rearrange("p q z -> p (q z)"), A.Exp,
                )
                esw = tmpQ.tile([128, B * H], F32, name="esw")
                nc.vector.tensor_reduce(esw[:], ew[:], Ax.X, Alu.add)
                rcw = tmpQ.tile([128, B * H], F32, name="rcw")
                nc.vector.reciprocal(rcw[:], esw[:])
                nc.vector.tensor_tensor(
                    pattn[:], ew[:],
                    rcw[:][:, :, None].broadcast_to([128, B * H, 128]),
                    Alu.mult,
                )

            # ============ stage G: batched final combine (replicated) ============
            # comb[p=i, b, h, j]:  i = (x%32)*4 + y//32,  j = (y%32)*4 + r,
            # h = x//32.  Partition p for core c covers (c%2)*64 + xl*4 + y2.
            with (
                tc.tile_pool(name="finp", bufs=2) as finp,
                tc.tile_pool(name="psF", bufs=2, space="PSUM") as psF,
            ):
                comb = finp.tile([128, B, H, 128], F32, name="comb")
                nc.vector.tensor_copy(
                    comb[:].rearrange("p b h z -> p (b h) z"), pattn[:]
                )
                # scg[p, b, h, r, y1] via 8 DMAs per r
                scg = finp.tile([128, B, H, 2, 32], F32, name="scg")
                for r in range(NC):
                    for c in range(CORES):
                        p0 = (c % 2) * 64
                        nc.sync.dma_start(
                            scg[p0 : p0 + 64, :, c // 2, r, :],
                            ag_out[r][c].rearrange(
                                "x y2 b y1 -> (x y2) b y1"
                            ),
                        )
                comb_v = comb[:].rearrange("p b h (y1 r4) -> p b h y1 r4", r4=4)
                for r in range(NC):
                    nc.vector.tensor_tensor(
                        comb_v[:, :, :, :, r], comb_v[:, :, :, :, r],
                        scg[:, :, :, r, :], Alu.add,
                    )
                # global min/max -> 1/(hi-lo) broadcast to all partitions
                mm = finp.tile([128, 2], F32, name="mm")
                cflat = comb[:].rearrange("p b h z -> p (b h z)")
                nc.vector.tensor_reduce(mm[:, 0:1], cflat, Ax.X, Alu.max)
                nc.vector.tensor_reduce(
                    mm[:, 1:2], cflat, Ax.X, Alu.min, negate=True
                )
                psf = psF.tile([128, 128], F32, name="psF")
                nc.tensor.transpose(psf[0:2, :], mm[:], ident32[:])
                hilo = finp.tile([2, 128], F32, name="hilo")
                nc.vector.tensor_copy(hilo[:], psf[0:2, :])
                hl2 = finp.tile([2, 1], F32, name="hl2")
                nc.vector.tensor_reduce(hl2[:], hilo[:], Ax.X, Alu.max)
                hl16 = finp.tile([2, 1], F16, name="hl16")
                nc.vector.tensor_copy(hl16[:], hl2[:])
                ones2 = finp.tile([2, 128], F16, name="ones2")
                nc.sync.dma_start(
                    ones2[:], ones_in[0:1, 0:128].to_broadcast([2, 128])
                )
                psr = psF.tile([128, 1], F32, name="psr")
                nc.tensor.matmul(psr[:], ones2[:], hl16[:], start=True, stop=True)
                rcpb = finp.tile([128, 1], F32, name="rcpb")
                nc.vector.reciprocal(rcpb[:], psr[:])

                # softmax over j per pane; lo/(hi-lo) shift is a row constant
                # -> softmax-invariant; comb*rcp is bounded so exp needs no
                # max shift
                nrm = finp.tile([128, B * H * 128], F32, name="nrm")
                nc.vector.tensor_scalar_mul(nrm[:], cflat, rcpb[:])
                ef = finp.tile([128, B * H, 128], F32, name="ef")
                nc.scalar.activation(
                    ef[:].rearrange("p q z -> p (q z)"), nrm[:], A.Exp
                )
                esums = finp.tile([128, B * H], F32, name="esums")
                nc.vector.tensor_reduce(esums[:], ef[:], Ax.X, Alu.add)
                recs = finp.tile([128, B * H], F32, name="recs")
                nc.vector.reciprocal(recs[:], esums[:])
                of = finp.tile([128, B * H, 128], F32, name="of")
                nc.vector.tensor_tensor(
                    of[:], ef[:],
                    recs[:][:, :, None].broadcast_to([128, B * H, 128]),
                    Alu.mult,
                )
                nc.sync.dma_start(
                    out[:].rearrange("b h i j -> i (b h) j"), of[:]
                )

    _split_multiwaits(nc)
    return nc


# ----------------------------------------------------------------------------
# Host-side input preparation (per core)
# ----------------------------------------------------------------------------


def prep_inputs(inputs):
    """inputs: dict of full numpy arrays as produced by setup_inputs().
    Returns (in_maps, npos): per-core input dicts + per-r positive counts."""
    f32 = np.float32
    f16 = np.float16
    te = np.ascontiguousarray(inputs["text_embeddings"], dtype=f32)  # [B, L, D]
    query = np.ascontiguousarray(inputs["query"], dtype=f32)
    key = np.ascontiguousarray(inputs["key"], dtype=f32)
    mask = inputs["mask"]
    Wtri = np.ascontiguousarray(inputs["Wtri"], dtype=f32)  # [385, 384, 385, 2]
    Vw = inputs["Vw"].astype(f32).reshape(2, D)

    def kt(a, s):  # [K, M] -> [128, K//128, M]
        K, M = a.shape
        assert K == s * 128
        return np.ascontiguousarray(a.reshape(s, 128, M).transpose(1, 0, 2))

    def rowsT(a):  # [B, L, D] -> [128, D//128, B*L] transposed k-tiled
        Dm = a.shape[-1]
        flat = a.reshape(-1, Dm).T  # [D, B*L]
        return np.ascontiguousarray(
            flat.reshape(Dm // 128, 128, flat.shape[1]).transpose(1, 0, 2)
        )

    def bias_t(b, s):  # [s*128] -> [128, s]
        return np.ascontiguousarray(b.reshape(s, 128).T)

    # per-r sign-sorted |Vw|-scaled text embeddings: ter[z, b, r, h']
    te_zbd = te.transpose(1, 0, 2)  # [L(z), B, D]
    ter = np.empty((L, B, NC, D), f32)
    npos = []
    for r in range(NC):
        order = np.argsort((Vw[r] <= 0).astype(np.int32), kind="stable")
        npos.append(int((Vw[r] > 0).sum()))
        ter[:, :, r, :] = te_zbd[:, :, order] * np.abs(Vw[r][order])
    npos = tuple(npos)

    common = {
        "teT": rowsT(te).astype(f16),
        "ter": np.ascontiguousarray(ter).astype(f16),
        "Wq": kt(inputs["Wq"].astype(f32), 6).astype(f16),
        "bq": bias_t(inputs["bq"].astype(f32), 6),
        "Wk": kt(inputs["Wk"].astype(f32), 6).astype(f16),
        "bk": bias_t(inputs["bk"].astype(f32), 6),
        "qryT": rowsT(query).astype(f16),
        "keyT": rowsT(key).astype(f16),
        "pmask": np.ascontiguousarray(
            np.where(mask == 0, np.float32(-1e9), np.float32(0.0)).transpose(1, 0, 2)
        ),
        "Vb": inputs["Vb"].astype(f32).reshape(2, 1),
        "ones16": np.ones((1, BL), f16),
    }
    for nm in ("h", "m", "t"):
        common[f"W{nm}1"] = kt(inputs[f"W{nm}1"].astype(f32), 6).astype(f16)
        common[f"b{nm}1"] = bias_t(inputs[f"b{nm}1"].astype(f32), 3)
        common[f"W{nm}2"] = kt(inputs[f"W{nm}2"].astype(f32), 3).astype(f16)
        common[f"b{nm}2"] = bias_t(inputs[f"b{nm}2"].astype(f32), 3)

    # T-correction weights: WT[j, k, r] = Wtri[384, k, j, r]
    WT = np.ascontiguousarray(Wtri[384].transpose(1, 0, 2))  # [385, 384, 2]
    common["WbiT"] = np.ascontiguousarray(
        WT[:HD].reshape(3, 128, HD, 2).transpose(1, 0, 3, 2)
    ).astype(np.float16)  # [128, 3, 2, 384]
    common["Wlast"] = np.ascontiguousarray(
        WT[HD].T.reshape(1, 2, HD)
    ).astype(np.float16)

    idx = np.arange(L)
    in_maps = []
    for c in range(CORES):
        m = dict(common)
        j0 = c * JC
        # W core slice; jr rows ordered [e2_r0, e2_r1, r0 jj0..47, r1 jj0..47]
        blk = Wtri[:HD, :, j0 : j0 + JC, :]  # [384 i, 384 k, 48 j, 2 r]
        w_rows = np.ascontiguousarray(
            blk.transpose(3, 2, 1, 0).reshape(NJR, 3, 128, HD)
        )  # [row, ks, k_p, i], r-major
        bj = Wtri[:HD, :, 384, :]  # [384 i, 384 k, 2 r]
        w0 = np.empty((2, 3, 128, HD), dtype=f32)
        for r in range(2):
            w0[r] = bj[:, :, r].T.reshape(3, 128, HD)
        # W1c0: [128 k_p, 3 ks, 2 jl, HD i]
        m["W1c0"] = np.ascontiguousarray(w0.transpose(2, 1, 0, 3)).astype(f16)
        # W1c: [ch, 128 k_p, 3 ks, 4 jl, HD i]
        wc = w_rows.reshape(NCH, W_CHUNK, 3, 128, HD)
        m["W1c"] = np.ascontiguousarray(wc.transpose(0, 3, 2, 1, 4)).astype(f16)

        # softmax-z additive masks for this core's x chunk: [y(128), xl, z]
        xs = c * XL + np.arange(XL)
        zz = idx[None, None, :]
        yy = idx[:, None, None]
        xx = xs[None, :, None]
        bad = (zz > yy) | (zz < xx)  # [y, xl, z]
        m["madd"] = np.ascontiguousarray(np.where(bad, -1e6, 1e6).astype(f32))
        in_maps.append(m)
    return in_maps, npos


_CACHE = {}


def _get_built(npos, debug=False):
    key = ("nc", npos, debug)
    if key not in _CACHE:
        _CACHE[key] = build(npos, debug=debug)
    return _CACHE[key]


def run(inputs, debug=False, trace=False):
    from concourse.bass_utils import run_bass_kernel_spmd

    in_maps, npos = prep_inputs(inputs)
    nc = _get_built(npos, debug=debug)
    res = run_bass_kernel_spmd(
        nc, in_maps, list(range(CORES)), trace=trace
    )
    return res


def kernel(**inputs):
    res = run(inputs, debug=False)
    return np.ascontiguousarray(res.results[0]["out"])


if __name__ == "__main__":
    nc = build((384, 384), debug=False)
    print("build OK")
